# revision 1
# baseline (speedup 1.0000x reference)
"""Multi-head attention kernel for Trainium2 (Bass/Tile), 8-core data-parallel.

Problem: B=1024 batches of F=128 tokens, D=128 features, H=8 heads, dh=16.
  out = softmax(X Wq (X Wk)^T / sqrt(D)) (X Wv) + X Wr   (per head, concat)

Per-core structure (128 batches):
  - Host pre-transposes X to XT [D, B, F] bf16 (no on-device transpose,
    halved input DMA). All matmuls bf16 (fp32 matmul is 4x slower on PE).
  - Wq/Wk host-padded into A (heads 0-3) / B (heads 4-7) tiles with each
    head's 16 cols at a 32-aligned offset + 16 zero cols -> score matmuls
    are K=32 row-tiled matmuls at legal 32-aligned bases.
  - PSUM same-bank rule: matmuls with different row bases must not share a
    PSUM bank. Scores go to two 2-bank tiles X (bases 0/32 = head pairs
    {0,4},{1,5}) and Y (bases 64/96 = {2,6},{3,7}), one base per bank.
  - exp on ScalarE: two strided [128,512] instructions per batch (X then Y),
    scale=1/sqrt(D) fused, bf16 out feeds attn@V as stationary operand.
    Softmax max-subtraction skipped (|scores|/sqrt(D) < ~1).
  - attn@V: lhsT = expT_h [k,q], rhs = V_h; each head also emits its
    softmax denominator via an N=1 ones-vector matmul that reuses the same
    stationary expT. PSUM banks are split by LIFETIME (all base-0 writers):
    a V+denoms pair bank and an attn+R pair bank, so the pair-end tail only
    chains through recip, not through mul/add -> ~16% faster schedule.
  - Tail on VectorE at pair granularity: per-batch recip(denoms), then
    out = attn_unnorm * recip_broadcast + R (the reciprocal is broadcast
    across each head's 16 columns via a step-0 access pattern).
  - Output staged [F, B, E]; host transposes back.
"""

import numpy as np
import ml_dtypes

import concourse.bass as bass
import concourse.mybir as mybir
import concourse.tile as tile
from concourse import bacc
from concourse.bass_utils import run_bass_kernel_spmd

BF16 = ml_dtypes.bfloat16

N_CORES = 8
B, F, D = 1024, 128, 128
H, DH = 8, 16
BPC = B // N_CORES   # 128 batches per core
GIO = 8              # batches per IO wave (DMA granularity)
PAIR = 2             # batches per projection/tail pair
VCOLS = H * (DH + 1)  # 136
# vra_ps per-batch layout (512 f32 = one PSUM bank):
#   [V' 0:136 | R 136:264 | attn-out 264:400 | pad 400:512]
SCALE = 1.0 / float(D) ** 0.5
# et column-block order: X tile = heads 0,4,1,5; Y tile = heads 2,6,3,7
HORD = [0, 4, 1, 5, 2, 6, 3, 7]


def build_kernel(nc: bass.Bass):
    f32 = mybir.dt.float32
    bf16 = mybir.dt.bfloat16

    xt = nc.dram_tensor("xt", [D, BPC, F], bf16, kind="ExternalInput")
    # [WqA | WqB | WkA | WkB], each [D, 128], heads at 32-aligned cols
    wqk = nc.dram_tensor("wqk", [D, 4 * D], bf16, kind="ExternalInput")
    # [Wv (128) | Wr (128)] - plain, no padding
    wvr = nc.dram_tensor("wvr", [D, 2 * D], bf16, kind="ExternalInput")
    out = nc.dram_tensor("out", [F, BPC, D], f32, kind="ExternalOutput")

    with tile.TileContext(nc) as tc:
        with (
            tc.tile_pool(name="singles", bufs=1) as singles,
            tc.tile_pool(name="xtp", bufs=2) as xtp,
            tc.tile_pool(name="qksb", bufs=2) as qksb,
            tc.tile_pool(name="etp", bufs=2) as etp,
            tc.tile_pool(name="vp", bufs=3) as vp,
            tc.tile_pool(name="smalls", bufs=3) as smalls,
            tc.tile_pool(name="outp", bufs=2) as outp,
            tc.tile_pool(name="qkps", bufs=2, space="PSUM") as qkps_pool,
            tc.tile_pool(name="scxp", bufs=1, space="PSUM") as scxp_pool,
            tc.tile_pool(name="scyp", bufs=1, space="PSUM") as scyp_pool,
            tc.tile_pool(name="vdps", bufs=1, space="PSUM") as vdps_pool,
            tc.tile_pool(name="arps", bufs=1, space="PSUM") as arps_pool,
        ):
            wqk_sb = singles.tile([D, 4 * D], bf16)
            wvr_sb = singles.tile([D, 2 * D], bf16)
            ones_sb = singles.tile([D, 1], bf16)
            nc.vector.memset(ones_sb, 1.0)
            nc.sync.dma_start(out=wqk_sb, in_=wqk[:, :])
            nc.sync.dma_start(out=wvr_sb, in_=wvr[:, :])

            for w in range(BPC // GIO):  # 16 IO waves
                xtw = xtp.tile([D, GIO * F], bf16)
                nc.sync.dma_start(out=xtw, in_=xt[:, w * GIO:(w + 1) * GIO, :])
                ow = outp.tile([F, GIO * D], f32)

                def emit_qk(gbl):
                    # QT/KT projection for one batch: [QA|QB|KA|KB] x 128
                    xtb = xtw[:, gbl * F:(gbl + 1) * F]
                    qk_ps = qkps_pool.tile([D, 4 * F], f32)  # 1 bank
                    for i in range(4):
                        nc.tensor.matmul(
                            qk_ps[:, i * F:(i + 1) * F],
                            lhsT=wqk_sb[:, i * D:(i + 1) * D],
                            rhs=xtb,
                            start=True, stop=True,
                        )
                    qk_sb = qksb.tile([D, 4 * F], bf16)
                    nc.vector.tensor_copy(qk_sb, qk_ps)
                    return qk_sb

                # software-pipeline qk one pair ahead within the wave
                qk_pend = [emit_qk(0), emit_qk(1)]

                for p in range(GIO // PAIR):  # 4 pairs per wave
                    qk_sbs = qk_pend
                    qk_pend = []

                    # ---- V / R projections for both batches ----
                    # VD bank: V(b0) 0:128 | V(b1) 128:256 | denoms 256:272
                    # AR bank: A(b0) 0:128 | A(b1) 128:256 | R(b0) 256:384
                    #          | R(b1) 384:512   (all writers base-0)
                    vd_ps = vdps_pool.tile([F, 512], f32)
                    ar_ps = arps_pool.tile([F, 512], f32)
                    for b in range(PAIR):
                        gb = p * PAIR + b
                        xtb = xtw[:, gb * F:(gb + 1) * F]
                        nc.tensor.matmul(
                            vd_ps[:, b * D:(b + 1) * D],
                            lhsT=xtb, rhs=wvr_sb[:, 0:D],
                            start=True, stop=True,
                        )
                        nc.tensor.matmul(
                            ar_ps[:, 2 * D + b * D:2 * D + (b + 1) * D],
                            lhsT=xtb, rhs=wvr_sb[:, D:2 * D],
                            start=True, stop=True,
                        )
                    v_sb = vp.tile([F, PAIR * D], bf16)
                    nc.vector.tensor_copy(v_sb, vd_ps[:, 0:PAIR * D])
                    rc_pair = smalls.tile([F, PAIR * H], f32, tag="rc")
                    et_sbs = []

                    for b in range(PAIR):
                        gb = p * PAIR + b
                        qk_sb = qk_sbs[b]
                        qtA = qk_sb[:, 0 * F:1 * F]
                        qtB = qk_sb[:, 1 * F:2 * F]
                        ktA = qk_sb[:, 2 * F:3 * F]
                        ktB = qk_sb[:, 3 * F:4 * F]

                        # ---- scores into X (bases 0/32), Y (bases 64/96);
                        # emit all X heads, then Y, so exp-X can start while
                        # PE fills Y ----
                        sc_x = scxp_pool.tile([F, 1024], f32)  # 2 banks
                        sc_y = scyp_pool.tile([F, 1024], f32)  # 2 banks
                        for h in HORD:
                            qt = qtA if h < 4 else qtB
                            kt = ktA if h < 4 else ktB
                            s = (h % 4) * 32
                            sc = sc_x if (h % 4) < 2 else sc_y
                            col = ((h % 4) % 2) * 512 + (h // 4) * F
                            nc.tensor.matmul(
                                sc[:, col:col + F],
                                lhsT=kt[s:s + 32, :],
                                rhs=qt[s:s + 32, :],
                                start=True, stop=True,
                                tile_position=(s, 0),
                            )

                        # ---- exp: one strided instr per sc tile ----
                        et_sb = etp.tile([F, H * F], bf16)
                        for t_i, sc in enumerate((sc_x, sc_y)):
                            sc3 = sc.rearrange("p (bk c) -> p bk c", bk=2)
                            nc.scalar.activation(
                                et_sb[:, t_i * 512:(t_i + 1) * 512],
                                sc3[:, :, 0:2 * F],
                                mybir.ActivationFunctionType.Exp,
                                scale=SCALE,
                            )

                        et_sbs.append(et_sb)

                    # ---- attn @ V for both batches, emitted after BOTH
                    # batches' scores (in-order PE queue never blocks the
                    # next scores behind an exp-gated attnV). Each head also
                    # emits its softmax denominator via a ones-vector matmul
                    # reusing the same stationary expT. ----
                    for b in range(PAIR):
                        gb = p * PAIR + b
                        et_sb = et_sbs[b]
                        for h in range(H):
                            cbi = HORD.index(h)
                            lt = et_sb[:, cbi * F:(cbi + 1) * F]
                            nc.tensor.matmul(
                                ar_ps[:, b * D + h * DH:b * D + (h + 1) * DH],
                                lhsT=lt,
                                rhs=v_sb[:, (b * H + h) * DH:
                                         (b * H + h + 1) * DH],
                                start=True, stop=True,
                            )
                            nc.tensor.matmul(
                                vd_ps[:, 2 * D + b * H + h:
                                      2 * D + b * H + h + 1],
                                lhsT=lt, rhs=ones_sb,
                                start=True, stop=True,
                            )
                        if b == 0 and gb + PAIR < GIO:
                            qk_pend.append(emit_qk(gb + PAIR))

                    # one pair-level reciprocal (denoms are contiguous in VD;
                    # only the b1 recip gates the next pair's V-proj anyway)
                    nc.vector.reciprocal(
                        rc_pair, vd_ps[:, 2 * D:2 * D + PAIR * H]
                    )

                    # ---- pair-level tail: out = attn * recip_bcast + R ----
                    rc_bc = bass.AP(
                        tensor=rc_pair.tensor, offset=rc_pair.offset,
                        ap=[rc_pair.ap[0], [1, PAIR * H], [0, DH]],
                    )
                    o1 = smalls.tile([F, PAIR * D], f32)
                    nc.vector.tensor_mul(o1, ar_ps[:, 0:PAIR * D], rc_bc)
                    nc.vector.tensor_add(
                        ow[:, p * PAIR * D:(p + 1) * PAIR * D], o1,
                        ar_ps[:, PAIR * D:2 * PAIR * D],
                    )
                    # batch-1 qk of the next pair, after the tail
                    if (p + 1) * PAIR + 1 < GIO:
                        qk_pend.append(emit_qk((p + 1) * PAIR + 1))
                nc.sync.dma_start(out=out[:, w * GIO:(w + 1) * GIO, :], in_=ow)

    return nc


def _pad_qk(Wx: np.ndarray) -> np.ndarray:
    """[D, 128] -> [D, 256]: A/B groups of 4 heads at 32-aligned columns."""
    o = np.zeros((D, 2 * D), dtype=np.float32)
    for h in range(H):
        grp, s = divmod(h, 4)
        o[:, grp * D + s * 32:grp * D + s * 32 + DH] = Wx[:, h * DH:(h + 1) * DH]
    return o


def prep_in_maps(inputs_dict):
    inputs = np.asarray(inputs_dict["inputs"])
    W_query = np.asarray(inputs_dict["W_query"], dtype=np.float32)
    W_key = np.asarray(inputs_dict["W_key"], dtype=np.float32)
    W_value = np.asarray(inputs_dict["W_value"], dtype=np.float32)
    W_res = np.asarray(inputs_dict["W_res"], dtype=np.float32)

    xt_all = np.ascontiguousarray(inputs.transpose(2, 0, 1)).astype(BF16)
    wqk_np = np.concatenate([_pad_qk(W_query), _pad_qk(W_key)], axis=1).astype(BF16)
    wvr_np = np.concatenate([W_value, W_res], axis=1).astype(BF16)

    return [
        {
            "xt": np.ascontiguousarray(xt_all[:, c * BPC:(c + 1) * BPC, :]),
            "wqk": wqk_np,
            "wvr": wvr_np,
        }
        for c in range(N_CORES)
    ]


_COMPILED = {}


def _get_compiled():
    if "nc" not in _COMPILED:
        nc = bacc.Bacc(
            "TRN2", target_bir_lowering=False, debug=False, num_devices=N_CORES
        )
        build_kernel(nc)
        nc.compile()
        _COMPILED["nc"] = nc
    return _COMPILED["nc"]


def kernel(inputs, W_query, W_key, W_value, W_res, **kw):
    in_maps = prep_in_maps({
        "inputs": inputs, "W_query": W_query, "W_key": W_key,
        "W_value": W_value, "W_res": W_res,
    })
    nc = _get_compiled()
    res = run_bass_kernel_spmd(nc, in_maps, core_ids=list(range(N_CORES)))
    parts = [r["out"].transpose(1, 0, 2) for r in res.results]
    return np.concatenate(parts, axis=0)


if __name__ == "__main__":
    rng = np.random.default_rng(0)
    inp = {
        "inputs": rng.standard_normal((B, F, D)).astype(np.float32),
        "W_query": (rng.standard_normal((D, D)) * 0.05).astype(np.float32),
        "W_key": (rng.standard_normal((D, D)) * 0.05).astype(np.float32),
        "W_value": (rng.standard_normal((D, D)) * 0.05).astype(np.float32),
        "W_res": (rng.standard_normal((D, D)) * 0.05).astype(np.float32),
    }
    o = kernel(**inp)
    print("out shape", o.shape, o.dtype)



# revision 2
# speedup vs baseline: 1.4349x; 1.4349x over previous
"""Multi-head attention kernel for Trainium2 (Bass/Tile), 8-core data-parallel.

v4: evacuation-lean engine assignment with tile-granular dependency isolation.

Problem: B=1024 batches of F=128 tokens, D=128 features, H=8 heads, dh=16.
  out = softmax(X Wq (X Wk)^T / sqrt(D)) (X Wv) + X Wr   (per head, concat)

Per-core structure (128 batches):
  - Host pre-transposes X to XT [D, B, F] bf16. All matmuls bf16.
  - Q^T PACKED [128, F] (no padding). K zero-padded as TWO variants: KE
    (even heads at rows 32j..32j+16) / KO (odd heads at 32j+16..32j+32).
    Score head h (block j=h//2): lhsT=(KE|KO)[32j:32j+32], rhs=Q[32j:...],
    K=32 row tile at a 32-aligned base. Cuts Q/K evacuation 4F -> 3F.
  - Tile dependencies are whole-tile granular, so anything that must
    overlap lives in SEPARATE tiles:
      X-pool [F,1024] 2 banks (heads 0-3: block j bank j, 2 heads/bank)
      Y-pool [F,1024] 2 banks (heads 4-7)           -> exp_X(b) overlaps
      scores_Y(b) and scores_X(b+1) (pool gives per-batch tiles).
      qv_a/qv_b [D,512] 1 bank each: [Qpk|KE|KO|V] per batch parity;
      evacuated in ONE DVE copy [p,512] -> bf16 SBUF.
      rps [F,512] 1 bank: R projections, pair ping-pong (2x256). Only
      ungated ops touch it (R-proj write, early R-copy read).
      adp [F,512] 1 bank: attn 2x136 slots (128 attn + 8 denom), batch
      parity. Only post-exp ops touch it (attnV/denoms write, recip/mul
      read) so the exp gate can't leak into the proj/evac stream.
  - exp on ScalarE: TWO instrs per batch (X then Y), each [p,2,256] =
    512 elems, scale fused, bf16 out. Act ~1224ns/batch = the pace.
  - DVE ~1121ns/batch: evac 658 + Rcopy 392/pair + recip 142/pair +
    mul 392/pair (writes ow directly).
  - GpSimd (no PSUM port) does the final SBUF-only add: ow += R.
  - Output staged [F, B, D] f32; host transposes back.
"""

import numpy as np
import ml_dtypes

import concourse.bass as bass
import concourse.mybir as mybir
import concourse.tile as tile
from concourse import bacc
from concourse.bass_utils import run_bass_kernel_spmd

BF16 = ml_dtypes.bfloat16

N_CORES = 8
B, F, D = 1024, 128, 128
H, DH = 8, 16
BPC = B // N_CORES   # 128 batches per core
GIO = 8              # batches per IO wave (DMA granularity)
SCALE = 1.0 / float(D) ** 0.5


def build_kernel(nc: bass.Bass):
    f32 = mybir.dt.float32
    bf16 = mybir.dt.bfloat16

    xt = nc.dram_tensor("xt", [D, BPC, F], bf16, kind="ExternalInput")
    # [Wq packed | WKE | WKO], each [D,128]
    wqk = nc.dram_tensor("wqk", [D, 3 * D], bf16, kind="ExternalInput")
    wvr = nc.dram_tensor("wvr", [D, 2 * D], bf16, kind="ExternalInput")
    out = nc.dram_tensor("out", [F, BPC, D], f32, kind="ExternalOutput")

    with tile.TileContext(nc) as tc:
        with (
            tc.tile_pool(name="singles", bufs=1) as singles,
            tc.tile_pool(name="xtp", bufs=3) as xtp,
            tc.tile_pool(name="qkvp", bufs=6) as qkvp,
            tc.tile_pool(name="etp", bufs=6) as etp,
            tc.tile_pool(name="smalls", bufs=4) as smalls,
            tc.tile_pool(name="outp", bufs=3) as outp,
            tc.tile_pool(name="scx", bufs=1, space="PSUM") as scx_pool,
            tc.tile_pool(name="scy", bufs=1, space="PSUM") as scy_pool,
            tc.tile_pool(name="qvps", bufs=1, space="PSUM") as qvps_pool,
            tc.tile_pool(name="rps", bufs=1, space="PSUM") as rps_pool,
            tc.tile_pool(name="adps", bufs=1, space="PSUM") as adps_pool,
        ):
            wqk_sb = singles.tile([D, 3 * D], bf16)
            wvr_sb = singles.tile([D, 2 * D], bf16)
            ones_sb = singles.tile([D, 1], bf16)
            nc.vector.memset(ones_sb, 1.0)
            nc.sync.dma_start(out=wqk_sb, in_=wqk[:, :])
            nc.sync.dma_start(out=wvr_sb, in_=wvr[:, :])

            # persistent PSUM tiles
            qv_a = qvps_pool.tile([D, 512], f32)
            qv_b = qvps_pool.tile([D, 512], f32)
            qvp = [qv_a, qv_b]
            rps = rps_pool.tile([F, 512], f32)
            adp = adps_pool.tile([F, 512], f32)
            adpb = adp.rearrange("p (bk c) -> p bk c", bk=2)  # [p,2,256]

            xtw = [None, None]   # input wave tiles
            qkv = {}             # batch -> evacuated [Q|KE|KO|V] sbuf tile
            scxy = {}            # batch -> (X score tile, Y score tile)
            et = {}              # batch -> (et_X [p,512], et_Y) sbuf tiles
            rtmp = {}            # pair -> evacuated R sbuf tile
            ow = [None, None]    # output wave tiles

            def emit_in_dma(w):
                t = xtp.tile([D, GIO * F], bf16, tag="xtw")
                nc.sync.dma_start(out=t, in_=xt[:, w * GIO:(w + 1) * GIO, :])
                xtw[w % 2] = t

            def xtb(b):
                return xtw[(b // GIO) % 2][:, (b % GIO) * F:(b % GIO + 1) * F]

            def emit_projs(b):
                qvt = qvp[b % 2]
                for i in range(3):
                    nc.tensor.matmul(
                        qvt[:, i * D:(i + 1) * D],
                        lhsT=wqk_sb[:, i * D:(i + 1) * D],
                        rhs=xtb(b), start=True, stop=True,
                    )
                nc.tensor.matmul(
                    qvt[:, 3 * D:4 * D],
                    lhsT=xtb(b), rhs=wvr_sb[:, 0:D],
                    start=True, stop=True,
                )
                # R slot: pair-parity ping-pong within the R bank
                ro = ((b // 2) % 2) * 2 * D + (b % 2) * D
                nc.tensor.matmul(
                    rps[:, ro:ro + D],
                    lhsT=xtb(b), rhs=wvr_sb[:, D:2 * D],
                    start=True, stop=True,
                )

            def emit_evac(b):
                t = qkvp.tile([D, 512], bf16)
                nc.vector.tensor_copy(t, qvp[b % 2][:, :])
                qkv[b] = t

            def emit_scores(b):
                sb = qkv[b]
                tx = scx_pool.tile([F, 1024], f32, tag="sx")
                ty = scy_pool.tile([F, 1024], f32, tag="sy")
                scxy[b] = (tx, ty)
                for h in range(H):
                    j, o = divmod(h, 2)
                    t = tx if j < 2 else ty
                    nc.tensor.matmul(
                        t[:, (j % 2) * 512 + o * F:(j % 2) * 512 + (o + 1) * F],
                        lhsT=sb[:, (1 + o) * D:(2 + o) * D][j * 32:(j + 1) * 32, :],
                        rhs=sb[:, 0:D][j * 32:(j + 1) * 32, :],
                        start=True, stop=True,
                        tile_position=(j * 32, 0),
                    )

            def emit_exp(b):
                ts = []
                for t in scxy[b]:
                    t3 = t.rearrange("p (bk c) -> p bk c", bk=2)
                    e = etp.tile([F, 512], bf16, tag="et")
                    nc.scalar.activation(
                        e, t3[:, :, 0:2 * F],
                        mybir.ActivationFunctionType.Exp, scale=SCALE,
                    )
                    ts.append(e)
                et[b] = ts
                scxy.pop(b, None)

            def ethead(b, h):
                # et tile layout: X = blocks j=0,1 -> heads (0,1),(2,3)
                j, o = divmod(h, 2)
                t = et[b][0] if j < 2 else et[b][1]
                return t[:, ((j % 2) * 2 + o) * F:((j % 2) * 2 + o + 1) * F]

            def emit_denoms(b):
                for h in range(H):
                    nc.tensor.matmul(
                        adp[:, (b % 2) * 256 + 128 + h:(b % 2) * 256 + 129 + h],
                        lhsT=ethead(b, h), rhs=ones_sb, start=True, stop=True,
                    )

            def emit_attnv(b):
                sb = qkv[b]
                for h in range(H):
                    nc.tensor.matmul(
                        adp[:, (b % 2) * 256 + h * DH:
                            (b % 2) * 256 + (h + 1) * DH],
                        lhsT=ethead(b, h),
                        rhs=sb[:, 3 * D + h * DH:3 * D + (h + 1) * DH],
                        start=True, stop=True,
                    )

            def emit_rcopy(p):
                t = smalls.tile([F, 2 * D], bf16, tag="rt")
                nc.vector.tensor_copy(t, rps[:, (p % 2) * 2 * D:(p % 2 + 1) * 2 * D])
                rtmp[p] = t

            def emit_tail(p):
                # pair p = (2p, 2p+1): batch-parity attn/den slots in adp
                rc = smalls.tile([F, 2 * H], f32, tag="rc")
                nc.vector.reciprocal(rc, adpb[:, :, 128:128 + H])
                rc_bc = bass.AP(
                    tensor=rc.tensor, offset=rc.offset,
                    ap=[rc.ap[0], [1, 2 * H], [0, DH]],
                )
                w, g = divmod(2 * p, GIO)
                dst = ow[w % 2][:, g * D:(g + 2) * D]
                nc.vector.tensor_mul(dst, adpb[:, :, 0:128], rc_bc)
                nc.gpsimd.tensor_add(dst, dst, rtmp[p])
                rtmp.pop(p, None)

            def emit_out_dma(w):
                nc.sync.dma_start(
                    out=out[:, w * GIO:(w + 1) * GIO, :], in_=ow[w % 2]
                )

            # ---- software-pipelined main loop ----
            emit_in_dma(0)
            owt = outp.tile([F, GIO * D], f32)
            ow[0] = owt
            for b in range(-2, BPC + 2):
                if b >= 0 and b % GIO == 0 and b // GIO + 1 < BPC // GIO:
                    emit_in_dma(b // GIO + 1)
                if b >= 2 and (b - 2) % GIO == 0 and (b - 2) // GIO > 0:
                    owt = outp.tile([F, GIO * D], f32)
                    ow[((b - 2) // GIO) % 2] = owt
                if 0 <= b < BPC:
                    emit_scores(b)
                if b + 2 < BPC:
                    emit_projs(b + 2)
                    emit_evac(b + 2)
                if b >= 0 and b % 2 == 0 and b + 1 < BPC:
                    emit_rcopy(b // 2)
                if 0 <= b < BPC:
                    emit_exp(b)
                if 0 <= b - 1 < BPC:
                    emit_denoms(b - 1)
                    emit_attnv(b - 1)
                # tail for pair (b-2, b-1) right after its attnV emission
                if b >= 2 and b % 2 == 0:
                    emit_tail((b - 2) // 2)
                    if (b - 2) % GIO == GIO - 2:
                        emit_out_dma((b - 2) // GIO)
                qkv.pop(b - 2, None)
                et.pop(b - 2, None)

    return nc


def _prep_wqk(Wq: np.ndarray, Wk: np.ndarray) -> np.ndarray:
    """[Wq packed | KE | KO]: KE/KO zero-pad even/odd heads into 32-blocks."""
    ke = np.zeros((D, D), dtype=np.float32)
    ko = np.zeros((D, D), dtype=np.float32)
    for j in range(4):
        ke[:, 32 * j:32 * j + DH] = Wk[:, DH * 2 * j:DH * (2 * j + 1)]
        ko[:, 32 * j + DH:32 * j + 32] = Wk[:, DH * (2 * j + 1):DH * (2 * j + 2)]
    return np.concatenate([Wq, ke, ko], axis=1)


def prep_in_maps(inputs_dict):
    inputs = np.asarray(inputs_dict["inputs"])
    W_query = np.asarray(inputs_dict["W_query"], dtype=np.float32)
    W_key = np.asarray(inputs_dict["W_key"], dtype=np.float32)
    W_value = np.asarray(inputs_dict["W_value"], dtype=np.float32)
    W_res = np.asarray(inputs_dict["W_res"], dtype=np.float32)

    xt_all = np.ascontiguousarray(inputs.transpose(2, 0, 1)).astype(BF16)
    wqk_np = _prep_wqk(W_query, W_key).astype(BF16)
    wvr_np = np.concatenate([W_value, W_res], axis=1).astype(BF16)

    return [
        {
            "xt": np.ascontiguousarray(xt_all[:, c * BPC:(c + 1) * BPC, :]),
            "wqk": wqk_np,
            "wvr": wvr_np,
        }
        for c in range(N_CORES)
    ]


_COMPILED = {}


def _get_compiled():
    if "nc" not in _COMPILED:
        nc = bacc.Bacc(
            "TRN2", target_bir_lowering=False, debug=False, num_devices=N_CORES
        )
        build_kernel(nc)
        nc.compile()
        _COMPILED["nc"] = nc
    return _COMPILED["nc"]


def kernel(inputs, W_query, W_key, W_value, W_res, **kw):
    in_maps = prep_in_maps({
        "inputs": inputs, "W_query": W_query, "W_key": W_key,
        "W_value": W_value, "W_res": W_res,
    })
    nc = _get_compiled()
    res = run_bass_kernel_spmd(nc, in_maps, core_ids=list(range(N_CORES)))
    parts = [r["out"].transpose(1, 0, 2) for r in res.results]
    return np.concatenate(parts, axis=0)


if __name__ == "__main__":
    rng = np.random.default_rng(0)
    inp = {
        "inputs": rng.standard_normal((B, F, D)).astype(np.float32),
        "W_query": (rng.standard_normal((D, D)) * 0.05).astype(np.float32),
        "W_key": (rng.standard_normal((D, D)) * 0.05).astype(np.float32),
        "W_value": (rng.standard_normal((D, D)) * 0.05).astype(np.float32),
        "W_res": (rng.standard_normal((D, D)) * 0.05).astype(np.float32),
    }
    o = kernel(**inp)

    X, Wq, Wk, Wv, Wr = (inp["inputs"], inp["W_query"], inp["W_key"],
                         inp["W_value"], inp["W_res"])
    def proj(x, w):
        y = np.einsum('bfd,de->bfe', x, w)
        return y.reshape(B, F, H, DH).transpose(0, 2, 1, 3)
    Q, K, V, R = proj(X, Wq), proj(X, Wk), proj(X, Wv), proj(X, Wr)
    s = np.einsum('bhqd,bhkd->bhqk', Q, K) * SCALE
    a = np.exp(s); a = a / a.sum(-1, keepdims=True)
    ref = (np.einsum('bhqk,bhkd->bhqd', a, V) + R)
    ref = ref.transpose(0, 2, 1, 3).reshape(B, F, D)
    rel = np.linalg.norm(o - ref) / np.linalg.norm(ref)
    print("out shape", o.shape, o.dtype, "rel err", rel)


# revision 3
# speedup vs baseline: 1.7069x; 1.1895x over previous
"""Multi-head attention kernel for Trainium2 (Bass/Tile), 8-core data-parallel.

v6: pair-packed score tiles -> one contiguous exp instr per pool per PAIR
(Act 1038ns/batch), evacuation-lean engine assignment, tile-granular deps.

Problem: B=1024 batches of F=128 tokens, D=128 features, H=8 heads, dh=16.
  out = softmax(X Wq (X Wk)^T / sqrt(D)) (X Wv) + X Wr   (per head, concat)

Per-core structure (128 batches):
  - Host pre-transposes X to XT [D, B, F] bf16. All matmuls bf16.
  - Q^T PACKED [128, F] (no padding). K zero-padded as TWO variants: KE
    (even heads at rows 32j..32j+16) / KO (odd heads at 32j+16..32j+32).
    Score head h (block j=h//2): lhsT=(KE|KO)[32j:32j+32], rhs=Q[32j:...],
    K=32 row tile at a 32-aligned base. Cuts Q/K evacuation 4F -> 3F.
  - Tile dependencies are whole-tile granular, so anything that must
    overlap lives in SEPARATE tiles:
      X-pool [F,1024] 2 banks (heads 0-3: block j bank j, 2 heads/bank)
      Y-pool [F,1024] 2 banks (heads 4-7)           -> exp_X(b) overlaps
      scores_Y(b) and scores_X(b+1) (pool gives per-batch tiles).
      qv_a/qv_b [D,512] 1 bank each: [Qpk|KE|KO|V] per batch parity;
      evacuated in ONE DVE copy [p,512] -> bf16 SBUF.
      rps [F,512] 1 bank: R projections, pair ping-pong (2x256). Only
      ungated ops touch it (R-proj write, early R-copy read).
      adp [F,512] 1 bank: attn 2x136 slots (128 attn + 8 denom), batch
      parity. Only post-exp ops touch it (attnV/denoms write, recip/mul
      read) so the exp gate can't leak into the proj/evac stream.
  - exp on ScalarE: TWO instrs per batch (X then Y), each [p,2,256] =
    512 elems, scale fused, bf16 out. Act ~1224ns/batch = the pace.
  - DVE ~1121ns/batch: evac 658 + Rcopy 392/pair + recip 142/pair +
    mul 392/pair (writes ow directly).
  - GpSimd (no PSUM port) does the final SBUF-only add: ow += R.
  - Output staged [F, B, D] f32; host transposes back.
"""

import numpy as np
import ml_dtypes

import concourse.bass as bass
import concourse.mybir as mybir
import concourse.tile as tile
from concourse import bacc
from concourse.bass_utils import run_bass_kernel_spmd

BF16 = ml_dtypes.bfloat16

N_CORES = 8
B, F, D = 1024, 128, 128
H, DH = 8, 16
BPC = B // N_CORES   # 128 batches per core
GIO = 8              # batches per IO wave (DMA granularity)
SCALE = 1.0 / float(D) ** 0.5


def build_kernel(nc: bass.Bass):
    f32 = mybir.dt.float32
    bf16 = mybir.dt.bfloat16

    xt = nc.dram_tensor("xt", [D, BPC, F], bf16, kind="ExternalInput")
    # [Wq packed | WKE | WKO], each [D,128]
    wqk = nc.dram_tensor("wqk", [D, 3 * D], bf16, kind="ExternalInput")
    wvr = nc.dram_tensor("wvr", [D, 2 * D], bf16, kind="ExternalInput")
    out = nc.dram_tensor("out", [F, BPC, D], f32, kind="ExternalOutput")

    with tile.TileContext(nc) as tc:
        with (
            tc.tile_pool(name="singles", bufs=1) as singles,
            tc.tile_pool(name="xtp", bufs=3) as xtp,
            tc.tile_pool(name="qkvp", bufs=10) as qkvp,
            tc.tile_pool(name="etp", bufs=6) as etp,
            tc.tile_pool(name="smalls", bufs=4) as smalls,
            tc.tile_pool(name="outp", bufs=5) as outp,
            tc.tile_pool(name="scx", bufs=1, space="PSUM") as scx_pool,
            tc.tile_pool(name="scy", bufs=1, space="PSUM") as scy_pool,
            tc.tile_pool(name="qvps", bufs=1, space="PSUM") as qvps_pool,
            tc.tile_pool(name="rps", bufs=1, space="PSUM") as rps_pool,
            tc.tile_pool(name="adps", bufs=1, space="PSUM") as adps_pool,
        ):
            wqk_sb = singles.tile([D, 3 * D], bf16)
            wvr_sb = singles.tile([D, 2 * D], bf16)
            ones_sb = singles.tile([D, 1], bf16)
            nc.vector.memset(ones_sb, 1.0)
            nc.sync.dma_start(out=wqk_sb, in_=wqk[:, :])
            nc.sync.dma_start(out=wvr_sb, in_=wvr[:, :])

            # persistent PSUM tiles
            qv_a = qvps_pool.tile([D, 512], f32)
            qv_b = qvps_pool.tile([D, 512], f32)
            qvp = [qv_a, qv_b]
            rps = rps_pool.tile([F, 512], f32)
            adp = adps_pool.tile([F, 512], f32)
            adpb = adp.rearrange("p (bk c) -> p bk c", bk=2)  # [p,2,256]

            xtw = [None, None, None]   # input wave tiles (3-rotation)
            xtw_first = None     # 2-batch startup tile
            qkv = {}             # batch -> evacuated [Q|KE|KO|V] sbuf tile
            scxy = {}            # batch -> (X score tile, Y score tile)
            et = {}              # batch -> (et_X [p,512], et_Y) sbuf tiles
            rtmp = {}            # pair -> evacuated R sbuf tile
            ow = [None, None, None, None]   # half-wave output tiles

            def emit_in_dma(w):
                t = xtp.tile([D, GIO * F], bf16, tag="xtw")
                nc.sync.dma_start(out=t, in_=xt[:, w * GIO:(w + 1) * GIO, :])
                xtw[w % 3] = t

            def xtb(b):
                if b < 2:
                    return xtw_first[:, b * F:(b + 1) * F]
                return xtw[(b // GIO) % 3][:, (b % GIO) * F:(b % GIO + 1) * F]

            def emit_projs(b):
                qvt = qvp[b % 2]
                for i in range(3):
                    nc.tensor.matmul(
                        qvt[:, i * D:(i + 1) * D],
                        lhsT=wqk_sb[:, i * D:(i + 1) * D],
                        rhs=xtb(b), start=True, stop=True,
                    )
                nc.tensor.matmul(
                    qvt[:, 3 * D:4 * D],
                    lhsT=xtb(b), rhs=wvr_sb[:, 0:D],
                    start=True, stop=True,
                )
                # R slot: pair-parity ping-pong within the R bank
                ro = ((b // 2) % 2) * 2 * D + (b % 2) * D
                nc.tensor.matmul(
                    rps[:, ro:ro + D],
                    lhsT=xtb(b), rhs=wvr_sb[:, D:2 * D],
                    start=True, stop=True,
                )

            def emit_evac(b):
                t = qkvp.tile([D, 512], bf16)
                nc.vector.tensor_copy(t, qvp[b % 2][:, :])
                qkv[b] = t

            def alloc_sc(p):
                tx = scx_pool.tile([F, 1024], f32, tag="sx")
                ty = scy_pool.tile([F, 1024], f32, tag="sy")
                scxy[p] = (tx, ty)

            def emit_scores(b, grp):
                # pair tile: head (j,o) of batch b at col
                # (j%2)*512 + (b%2)*256 + o*128  (bank j%2, base 32j).
                # grp 0 = X pool (heads 0-3), grp 1 = Y pool (heads 4-7):
                # emitted pool-blocked so the X chain never waits exp_Y.
                sb = qkv[b]
                t = scxy[b // 2][grp]
                for h in range(4 * grp, 4 * grp + 4):
                    j, o = divmod(h, 2)
                    c = (j % 2) * 512 + (b % 2) * 256 + o * F
                    nc.tensor.matmul(
                        t[:, c:c + F],
                        lhsT=sb[:, (1 + o) * D:(2 + o) * D][j * 32:(j + 1) * 32, :],
                        rhs=sb[:, 0:D][j * 32:(j + 1) * 32, :],
                        start=True, stop=True,
                        tile_position=(j * 32, 0),
                    )

            def emit_exp(p):
                ts = []
                for t in scxy[p]:
                    e = etp.tile([F, 1024], bf16, tag="et")
                    nc.scalar.activation(
                        e, t[:, :],
                        mybir.ActivationFunctionType.Exp, scale=SCALE,
                    )
                    ts.append(e)
                et[p] = ts
                scxy.pop(p, None)

            def ethead(b, h):
                j, o = divmod(h, 2)
                t = et[b // 2][0] if j < 2 else et[b // 2][1]
                c = (j % 2) * 512 + (b % 2) * 256 + o * F
                return t[:, c:c + F]

            def emit_denoms(b):
                for h in range(H):
                    nc.tensor.matmul(
                        adp[:, (b % 2) * 256 + 128 + h:(b % 2) * 256 + 129 + h],
                        lhsT=ethead(b, h), rhs=ones_sb, start=True, stop=True,
                    )

            def emit_attnv(b):
                sb = qkv[b]
                for h in range(H):
                    nc.tensor.matmul(
                        adp[:, (b % 2) * 256 + h * DH:
                            (b % 2) * 256 + (h + 1) * DH],
                        lhsT=ethead(b, h),
                        rhs=sb[:, 3 * D + h * DH:3 * D + (h + 1) * DH],
                        start=True, stop=True,
                    )

            def emit_rcopy(p):
                t = smalls.tile([F, 2 * D], bf16, tag="rt")
                nc.vector.tensor_copy(t, rps[:, (p % 2) * 2 * D:(p % 2 + 1) * 2 * D])
                rtmp[p] = t

            def emit_tail(p):
                # pair p = (2p, 2p+1): batch-parity attn/den slots in adp
                rc = smalls.tile([F, 2 * H], f32, tag="rc")
                nc.vector.reciprocal(rc, adpb[:, :, 128:128 + H])
                rc_bc = bass.AP(
                    tensor=rc.tensor, offset=rc.offset,
                    ap=[rc.ap[0], [1, 2 * H], [0, DH]],
                )
                h4, g = divmod(2 * p, GIO // 2)
                dst = ow[h4 % 4][:, g * D:(g + 2) * D]
                nc.vector.tensor_mul(dst, adpb[:, :, 0:128], rc_bc)
                nc.gpsimd.tensor_add(dst, dst, rtmp[p])
                rtmp.pop(p, None)

            def emit_out_dma(h4):
                # half-wave granularity: 4 batches per output DMA
                nc.sync.dma_start(
                    out=out[:, h4 * 4:(h4 + 1) * 4, :], in_=ow[h4 % 4]
                )

            # ---- software-pipelined main loop ----
            # 2-batch startup DMA so the pipeline fills fast, then full waves
            xtw_first = xtp.tile([D, 2 * F], bf16, tag="xtf")
            nc.sync.dma_start(out=xtw_first, in_=xt[:, 0:2, :])
            emit_in_dma(0)
            owt = outp.tile([F, 4 * D], f32)
            ow[0] = owt
            emit_in_dma(1)
            NP = BPC // 2
            for s in range(-3, NP + 1):
                b0, b1 = 2 * s, 2 * s + 1
                if s >= 0 and b0 % GIO == 0 and b0 // GIO + 2 < BPC // GIO:
                    emit_in_dma(b0 // GIO + 2)
                if s >= 1 and (b0 - 2) % 4 == 0:
                    owt = outp.tile([F, 4 * D], f32)
                    ow[((b0 - 2) // 4) % 4] = owt
                # rcopy first: frees the R pair-bank before this slot's R projs
                if -1 <= s < NP - 1:
                    emit_rcopy(s + 1)
                # projections + evacuations THREE pairs ahead so queued evacs
                # clear the serial DVE before this slot's exp-gated tail
                if b0 + 6 < BPC:
                    emit_projs(b0 + 6)
                    emit_evac(b0 + 6)
                if b1 + 6 < BPC:
                    emit_projs(b1 + 6)
                    emit_evac(b1 + 6)
                if 0 <= s < NP:
                    alloc_sc(s)
                    emit_scores(b0, 0)
                    emit_scores(b1, 0)
                    emit_scores(b0, 1)
                    emit_scores(b1, 1)
                if 0 <= s < NP:
                    emit_exp(s)
                if s >= 1:
                    emit_denoms(b0 - 2)
                    emit_attnv(b0 - 2)
                    emit_denoms(b1 - 2)
                    emit_attnv(b1 - 2)
                    emit_tail(s - 1)
                    if (b0 - 2) % 4 == 2:
                        emit_out_dma((b0 - 2) // 4)
                qkv.pop(b0 - 4, None)
                qkv.pop(b1 - 4, None)
                et.pop(s - 2, None)

    return nc


def _prep_wqk(Wq: np.ndarray, Wk: np.ndarray) -> np.ndarray:
    """[Wq packed | KE | KO]: KE/KO zero-pad even/odd heads into 32-blocks."""
    ke = np.zeros((D, D), dtype=np.float32)
    ko = np.zeros((D, D), dtype=np.float32)
    for j in range(4):
        ke[:, 32 * j:32 * j + DH] = Wk[:, DH * 2 * j:DH * (2 * j + 1)]
        ko[:, 32 * j + DH:32 * j + 32] = Wk[:, DH * (2 * j + 1):DH * (2 * j + 2)]
    return np.concatenate([Wq, ke, ko], axis=1)


def prep_in_maps(inputs_dict):
    inputs = np.asarray(inputs_dict["inputs"])
    W_query = np.asarray(inputs_dict["W_query"], dtype=np.float32)
    W_key = np.asarray(inputs_dict["W_key"], dtype=np.float32)
    W_value = np.asarray(inputs_dict["W_value"], dtype=np.float32)
    W_res = np.asarray(inputs_dict["W_res"], dtype=np.float32)

    xt_all = np.ascontiguousarray(inputs.transpose(2, 0, 1)).astype(BF16)
    wqk_np = _prep_wqk(W_query, W_key).astype(BF16)
    wvr_np = np.concatenate([W_value, W_res], axis=1).astype(BF16)

    return [
        {
            "xt": np.ascontiguousarray(xt_all[:, c * BPC:(c + 1) * BPC, :]),
            "wqk": wqk_np,
            "wvr": wvr_np,
        }
        for c in range(N_CORES)
    ]


_COMPILED = {}


def _get_compiled():
    if "nc" not in _COMPILED:
        nc = bacc.Bacc(
            "TRN2", target_bir_lowering=False, debug=False, num_devices=N_CORES
        )
        build_kernel(nc)
        nc.compile()
        _COMPILED["nc"] = nc
    return _COMPILED["nc"]


def kernel(inputs, W_query, W_key, W_value, W_res, **kw):
    in_maps = prep_in_maps({
        "inputs": inputs, "W_query": W_query, "W_key": W_key,
        "W_value": W_value, "W_res": W_res,
    })
    nc = _get_compiled()
    res = run_bass_kernel_spmd(nc, in_maps, core_ids=list(range(N_CORES)))
    parts = [r["out"].transpose(1, 0, 2) for r in res.results]
    return np.concatenate(parts, axis=0)


if __name__ == "__main__":
    rng = np.random.default_rng(0)
    inp = {
        "inputs": rng.standard_normal((B, F, D)).astype(np.float32),
        "W_query": (rng.standard_normal((D, D)) * 0.05).astype(np.float32),
        "W_key": (rng.standard_normal((D, D)) * 0.05).astype(np.float32),
        "W_value": (rng.standard_normal((D, D)) * 0.05).astype(np.float32),
        "W_res": (rng.standard_normal((D, D)) * 0.05).astype(np.float32),
    }
    o = kernel(**inp)

    X, Wq, Wk, Wv, Wr = (inp["inputs"], inp["W_query"], inp["W_key"],
                         inp["W_value"], inp["W_res"])
    def proj(x, w):
        y = np.einsum('bfd,de->bfe', x, w)
        return y.reshape(B, F, H, DH).transpose(0, 2, 1, 3)
    Q, K, V, R = proj(X, Wq), proj(X, Wk), proj(X, Wv), proj(X, Wr)
    s = np.einsum('bhqd,bhkd->bhqk', Q, K) * SCALE
    a = np.exp(s); a = a / a.sum(-1, keepdims=True)
    ref = (np.einsum('bhqk,bhkd->bhqd', a, V) + R)
    ref = ref.transpose(0, 2, 1, 3).reshape(B, F, D)
    rel = np.linalg.norm(o - ref) / np.linalg.norm(ref)
    print("out shape", o.shape, o.dtype, "rel err", rel)
